# revision 1
# baseline (speedup 1.0000x reference)
"""Trainium2 Bass kernel for nn_Conv2d_mvm (bit-streamed crossbar MVM conv).

Contract: kernel(**inputs) takes FULL unsharded inputs {x:[8,64,16,16] f32,
weight:[128,64,3,3] f32} and returns the FULL output [8,128,16,16] f32.

Sharding (8 cores): pixels P=2048 split 4 ways x crossbar-sign (pos/neg)
split 2 ways.  Core i: sign n=i//4, pixel quarter q=i%4 (512 pixels).
All cores run the identical SPMD program; sign is folded on the host
(out = (acc_pos - acc_neg) * delta * 2^-24).

Device algorithm per core (see host prep for exact defs):
  pass1 (PE, fp8):  col[128cols, 512pix] = xbars_tile.T @ bits_tile
                    per (r-block, col-tile t, stream s); col = exact ints <=192
  quant (ACT/DVE):  y = (+-85*2^s/64)*col + (+-2^(s+23))  -- fp32 RNE makes
                    y = +-2^s*(round_half_even(col*255/192) + 2^23) exactly
  unbias (DVE):     qp = y - (+-2^(s+23)) -> bf16 (exact, R<=255)
  pass2 (PE, bf16): acc[128 O, 512pix] += Wred.T @ qp  accumulated over
                    (r, s) in PSUM; Wred = block-diag slice weights 4^(7-sl)
"""

import numpy as np
import ml_dtypes
from contextlib import ExitStack

# ---- problem constants (hardcoded; must match the reference) ----
B, C, H, W = 8, 64, 16, 16
O, KH, KW = 128, 3, 3
PAD = 1
OH = OW = 16
L = C * KH * KW            # 576
XBAR = 64
SLICE_NUM = 8              # 16-bit weights / 2-bit slices
STREAM_NUM = 16            # 16-bit inputs / 1-bit streams
NSTATES = 3
W_FRAC = 12
I_FRAC = 12
XR = 9                     # ceil(576/64) row blocks
XC = 16                    # ceil(128*8/64) col blocks
P_TOTAL = B * OH * OW      # 2048
N_CORES = 8
P_CORE = P_TOTAL // 4      # 512 pixels per core (4-way pixel shard)

_COMPILED = {}


# ------------------------- host-side preprocessing -------------------------

def _bit_slice_weight(w_mag):
    w_int = np.clip(np.round(w_mag * 2.0**W_FRAC), 0, 2**16 - 1).astype(np.int32)
    shifts = (2 * np.arange(SLICE_NUM - 1, -1, -1)).astype(np.int32)
    slices = (w_int[:, :, None] >> shifts[None, None, :]) & NSTATES
    cout, l = w_mag.shape
    return slices.transpose(1, 0, 2).reshape(l, cout * SLICE_NUM).astype(np.float32)


def _prep_xbars(weight):
    """-> [2, 64k, 9r, 8t, 128m] fp8 device layout (m = 64*j + 8*cp + sl)."""
    wf = weight.reshape(O, L)
    pos = _bit_slice_weight(np.clip(wf, 0.0, None))
    neg = _bit_slice_weight(np.abs(np.clip(wf, None, 0.0)))
    wx = np.stack([pos, neg])                         # [2, 576, 1024]
    xb = wx.reshape(2, XR, XBAR, XC, XBAR).transpose(0, 1, 3, 2, 4)  # [2,r,c,k,o]
    # t covers c-blocks (2t, 2t+1)
    dev = xb.reshape(2, XR, 8, 2, XBAR, XBAR).transpose(0, 4, 1, 2, 3, 5)
    dev = dev.reshape(2, XBAR, XR, 8, 128)
    return np.ascontiguousarray(dev.astype(ml_dtypes.float8_e4m3))


def _prep_bits(x):
    """-> [64k, 9r, 16s, 2048p] fp8 (values 0/1)."""
    xp = np.pad(x, ((0, 0), (0, 0), (PAD, PAD), (PAD, PAD)))
    patches = np.stack([xp[:, :, di:di + OH, dj:dj + OW]
                        for di in range(KH) for dj in range(KW)], axis=2)
    feat = patches.reshape(B, L, OH * OW).transpose(0, 2, 1).reshape(P_TOTAL, L)
    x_int = np.clip(np.round(feat * 2.0**I_FRAC), -2**15, 2**15 - 1).astype(np.int32)
    x_u = np.where(x_int < 0, x_int + 2**16, x_int)
    shifts = np.arange(STREAM_NUM, dtype=np.int32)[:, None, None]
    bits = ((x_u[None] >> shifts) & 1).astype(np.float32)     # [16, 2048, 576]
    bits = bits.reshape(STREAM_NUM, P_TOTAL, XR, XBAR).transpose(3, 2, 0, 1)
    return np.ascontiguousarray(bits.astype(ml_dtypes.float8_e4m3))


def _prep_wred():
    """[128k, 8t, 128m] bf16 per-t slice-weight reduction matrices.

    For col-tile t, out row m = 16t + 8j + cp accumulates 4^(7-sl) * q[k]
    over k = 64j + 8cp + sl.  Full 128-row output (zeros elsewhere) because
    PE PSUM writes need base partition 0.
    """
    wred = np.zeros((128, 8, 128), np.float32)
    slice_w = 4.0 ** np.arange(SLICE_NUM - 1, -1, -1)
    for t in range(8):
        for j in range(2):
            for cp in range(8):
                for sl in range(8):
                    wred[64 * j + 8 * cp + sl, t, 16 * t + 8 * j + cp] = slice_w[sl]
    return wred.astype(ml_dtypes.bfloat16)


# ------------------------------ bass program ------------------------------

def _build_nc(act_frac=12):
    """One SPMD program for all 8 cores. act_frac/16 of quant ops go to ACT."""
    import concourse.bass as bass
    import concourse.mybir as mybir
    import concourse.tile as tile

    f8 = mybir.dt.float8e4
    bf16 = mybir.dt.bfloat16
    f32 = mybir.dt.float32

    nc = bass.Bass()
    bits_d = nc.dram_tensor("bits", [XBAR, XR, STREAM_NUM, P_CORE], f8,
                            kind="ExternalInput")
    xb_d = nc.dram_tensor("xbars", [XBAR, XR, 8, 128], f8, kind="ExternalInput")
    wred_d = nc.dram_tensor("wred", [128, 8, 128], bf16, kind="ExternalInput")
    out_d = nc.dram_tensor("acc_out", [128, P_CORE], f32, kind="ExternalOutput")

    with ExitStack() as ctx:
        tc = ctx.enter_context(tile.TileContext(nc))
        singles = ctx.enter_context(tc.tile_pool(name="singles", bufs=1))
        # separate pools per quantize-engine path so each tile has a fixed
        # producer/consumer engine pair (keeps sync waits <= 2 per inst)
        ypool_a = ctx.enter_context(tc.tile_pool(name="ya", bufs=3))
        ypool_b = ctx.enter_context(tc.tile_pool(name="yb", bufs=2))
        qpool_a = ctx.enter_context(tc.tile_pool(name="qpa", bufs=3))
        qpool_b = ctx.enter_context(tc.tile_pool(name="qpb", bufs=2))
        opool = ctx.enter_context(tc.tile_pool(name="osb", bufs=1))
        psq_pool_a = ctx.enter_context(tc.tile_pool(name="psqa", bufs=3,
                                                    space="PSUM"))
        psq_pool_b = ctx.enter_context(tc.tile_pool(name="psqb", bufs=2,
                                                    space="PSUM"))
        pacc_pool = ctx.enter_context(tc.tile_pool(name="pacc", bufs=1, space="PSUM"))

        bits_sb = singles.tile([XBAR, XR, STREAM_NUM, P_CORE], f8)
        nc.default_dma_engine.dma_start(out=bits_sb[:], in_=bits_d[:, :, :, :])
        xb_sb = singles.tile([XBAR, XR, 8, 128], f8)
        nc.default_dma_engine.dma_start(out=xb_sb[:], in_=xb_d[:, :, :, :])
        wred_sb = singles.tile([128, 8, 128], bf16)
        nc.default_dma_engine.dma_start(out=wred_sb[:], in_=wred_d[:, :, :])

        acc = pacc_pool.tile([128, P_CORE], f32)

        for r in range(XR):
            for t in range(8):
                # each quantize path stays on ONE engine (op1+op2) so every
                # instruction waits on at most {own-engine, PE} semaphores
                use_act = t < 3
                for s in range(STREAM_NUM):
                    sgn = -1.0 if s == 15 else 1.0
                    scale = float(np.float32(sgn * (85.0 / 64.0) * 2.0**s))
                    bias = float(np.float32(sgn * 2.0**(s + 23)))
                    if use_act:
                        psq = psq_pool_a.tile([128, P_CORE], f32, tag="psqa")
                    else:
                        psq = psq_pool_b.tile([128, P_CORE], f32, tag="psqb")
                    nc.tensor.matmul(psq[:, :], xb_sb[:, r, t, :],
                                     bits_sb[:, r, s, :], start=True, stop=True)
                    if use_act:
                        y = ypool_a.tile([128, P_CORE], f32, tag="ya")
                        nc.scalar.activation(
                            y[:, :], psq[:, :],
                            mybir.ActivationFunctionType.Copy,
                            bias=bias, scale=scale)
                        qp = qpool_a.tile([128, P_CORE], bf16, tag="qpa")
                        nc.scalar.activation(
                            qp[:, :], y[:, :],
                            mybir.ActivationFunctionType.Copy,
                            bias=-bias, scale=1.0)
                    else:
                        y = ypool_b.tile([128, P_CORE], f32, tag="yb")
                        nc.vector.tensor_scalar(
                            y[:, :], psq[:, :], scale, bias,
                            mybir.AluOpType.mult, mybir.AluOpType.add)
                        qp = qpool_b.tile([128, P_CORE], bf16, tag="qpb")
                        nc.vector.tensor_scalar(
                            qp[:, :], y[:, :], -bias, None, mybir.AluOpType.add)
                    nc.tensor.matmul(acc[:, :], wred_sb[:, t, :],
                                     qp[:, :],
                                     start=(r == 0 and t == 0 and s == 0),
                                     stop=(r == XR - 1 and t == 7
                                           and s == STREAM_NUM - 1))
        out_sb = opool.tile([128, P_CORE], f32)
        nc.vector.tensor_copy(out_sb[:, :], acc[:, :])
        nc.default_dma_engine.dma_start(out=out_d[:, :], in_=out_sb[:, :])

    _strip_own_engine_waits(nc, mybir)
    return nc


def _strip_own_engine_waits(nc, mybir):
    """Drop redundant same-engine semaphore waits (compute engines execute
    their queue serially, so ordering vs. their own past instructions is
    implicit).  Walrus's per-instruction sync structs have very few wait
    slots and reject Tile's extra own-engine waits."""
    eng_prefix = {
        "EngineType.PE": "PE",
        "EngineType.Activation": "Activation",
        "EngineType.DVE": "DVE",
        "EngineType.Pool": "Pool",
    }
    # The tail drain waits on every engine + every DMA queue, exceeding the
    # CTRL struct's wait slots.  All but the final DVE->DRAM chain are implied
    # transitively (DVE copy waits PE; PE waited ACT/DVE/input DMAs), so keep
    # only the DVE tick and the output DMA queue's semaphore.
    last_dma_sems = set()
    for f in nc.m.functions:
        for b in f.blocks:
            for inst in b.instructions:
                if type(inst).__name__ == "InstDMACopy" and inst.sync_info:
                    last_dma_sems = {str(w.ant_name)
                                     for w in (inst.sync_info.on_update or [])}
    for f in nc.m.functions:
        for b in f.blocks:
            for inst in b.instructions:
                si = getattr(inst, "sync_info", None)
                if (type(inst).__name__ == "InstDrain" and si and si.on_wait
                        and len(si.on_wait) > 2):
                    # output DMA completion implies the whole chain (it waits
                    # on DVE, which waits on PE, ...), so one wait suffices
                    kept = [w for w in si.on_wait
                            if str(w.ant_name) in last_dma_sems]
                    inst.sync_info = mybir.SyncInfo(
                        on_wait=kept, on_update=list(si.on_update or []))
    for f in nc.m.functions:
        for b in f.blocks:
            for inst in b.instructions:
                si = getattr(inst, "sync_info", None)
                if si is None or not si.on_wait:
                    continue
                pfx = eng_prefix.get(str(getattr(inst, "engine", None)))
                if pfx is None:
                    continue
                kept = [w for w in si.on_wait
                        if not str(w.ant_name).startswith(pfx + "_")]
                if len(kept) != len(si.on_wait):
                    inst.sync_info = mybir.SyncInfo(
                        on_wait=kept, on_update=list(si.on_update or []))


def _get_nc():
    if "nc" not in _COMPILED:
        _COMPILED["nc"] = _build_nc()
    return _COMPILED["nc"]


# ------------------------------- entry point -------------------------------

def _make_in_maps(x, weight):
    xbars = _prep_xbars(weight)      # [2, 64, 9, 8, 128]
    bits = _prep_bits(x)             # [64, 9, 16, 2048]
    wred = _prep_wred()
    in_maps = []
    for core in range(N_CORES):
        n, q = core // 4, core % 4
        in_maps.append({
            "bits": np.ascontiguousarray(bits[:, :, :, q * P_CORE:(q + 1) * P_CORE]),
            "xbars": np.ascontiguousarray(xbars[n]),
            "wred": wred,
        })
    return in_maps


def _postprocess(accs):
    """accs: list of 8 [128, 512] f32 arrays (core order) -> [8,128,16,16]."""
    acc_pos = np.concatenate([accs[q] for q in range(4)], axis=1)       # [128,2048]
    acc_neg = np.concatenate([accs[4 + q] for q in range(4)], axis=1)
    d32 = np.float32(192.0 / 255.0)
    out = ((acc_pos - acc_neg).astype(np.float32) * d32).astype(np.float32)
    out = out * np.float32(2.0**-24)
    amax = np.float32((2**15 - 1) / 2.0**12)
    out = np.clip(np.round(out * np.float32(4096.0)) / np.float32(4096.0),
                  -amax, amax).astype(np.float32)
    # out[o, p] with p = b*256 + i*16 + j  ->  [B, O, OH, OW]
    return np.ascontiguousarray(
        out.reshape(O, B, OH, OW).transpose(1, 0, 2, 3))


def run_on_hw(x, weight, trace=False):
    from concourse.bass_utils import run_bass_kernel_spmd
    nc = _get_nc()
    in_maps = _make_in_maps(np.asarray(x, np.float32), np.asarray(weight, np.float32))
    res = run_bass_kernel_spmd(nc, in_maps, list(range(N_CORES)), trace=trace)
    accs = [np.asarray(res.results[i]["acc_out"], np.float32)
            for i in range(N_CORES)]
    return _postprocess(accs), res


def kernel(x, weight):
    out, _ = run_on_hw(x, weight, trace=False)
    return out



# revision 9
# speedup vs baseline: 2.7087x; 2.7087x over previous
"""Trainium2 Bass kernel for nn_Conv2d_mvm (bit-streamed crossbar MVM conv).

Contract: kernel(**inputs) takes FULL unsharded inputs {x:[8,64,16,16] f32,
weight:[128,64,3,3] f32} and returns the FULL output [8,128,16,16] f32.

Sharding (8 cores): pixels P=2048 split 4 ways x crossbar-sign (pos/neg)
split 2 ways.  Core i: sign n=i//4, pixel quarter q=i%4 (512 pixels).
Sign is folded on the host: out = (acc_pos - acc_neg) * (192/255) * 2^-24.

Device algorithm per core:
  pass1 (PE, fp8):   col[128cols, 512pix] = xbars_tile.T @ bits_tile per
                     (r-block, col-tile t, stream s).  Tiles with K=64 are
                     row-packed two-at-a-time into the 128x128 PE array via
                     tile_position (0,0)/(64,0) -> both run concurrently.
  quant (ACT/DVE):   y_f16 = (85/64)*col + 1024.  fp16 RNE at the write
                     rounds q = round(col*255/192) exactly (q<=255, the
                     +1024 pins the exponent so ULP=1).  Constant scale and
                     bias for every tile.
  unbias (DVE):      qp_bf16 = y - 1024  (exact, single-ALU op -> fast mode)
  pass2 (PE, bf16):  acc[128 O, 512pix] += Wred[t,s].T @ qp, accumulated in
                     PSUM.  Wred folds both the per-slice weight 4^(7-sl)
                     and the per-stream weight +-2^s (exact powers of two).

Weight slices that are all-zero (high slices when |w| is small, e.g. the
default |w_int| < 2^10 case -> slices 0..2 empty) are dropped entirely:
n_act active slices -> T = n_act col-tiles of 128 instead of 8.
"""

import numpy as np
import ml_dtypes
from contextlib import ExitStack

# ---- problem constants (hardcoded; must match the reference) ----
B, C, H, W = 8, 64, 16, 16
O, KH, KW = 128, 3, 3
PAD = 1
OH = OW = 16
L = C * KH * KW            # 576
XBAR = 64
SLICE_NUM = 8              # 16-bit weights / 2-bit slices
STREAM_NUM = 16            # 16-bit inputs / 1-bit streams
NSTATES = 3
W_FRAC = 12
I_FRAC = 12
XR = 9                     # 576/64 row blocks (exact)
P_TOTAL = B * OH * OW      # 2048
N_CORES = 8
P_CORE = P_TOTAL // 4      # 512 pixels per core (4-way pixel shard)

# A-side r blocks (PE rows 0-63): pairs 0..3 plus the unpaired r=8.
# B-side r blocks (PE rows 64-127): pairs 4..7.
A_RS = (0, 1, 2, 3, 8)
B_RS = (4, 5, 6, 7)

_COMPILED = {}


# ------------------------- host-side preprocessing -------------------------

def _slice_cells(weight):
    """-> cells [2, L, O, 8] int (pos/neg, MSB-first slices), and sl_min."""
    wf = weight.reshape(O, L).astype(np.float64)
    pos = np.clip(np.round(np.clip(wf, 0.0, None) * 2.0**W_FRAC), 0, 2**16 - 1)
    neg = np.clip(np.round(np.abs(np.clip(wf, None, 0.0)) * 2.0**W_FRAC),
                  0, 2**16 - 1)
    w_int = np.stack([pos, neg]).astype(np.int64)          # [2, O, L]
    shifts = 2 * np.arange(SLICE_NUM - 1, -1, -1)
    cells = (w_int[:, :, :, None] >> shifts[None, None, None, :]) & NSTATES
    sl_min = 0
    for sl in range(SLICE_NUM):
        if cells[:, :, :, sl].any():
            sl_min = sl
            break
    return cells.transpose(0, 2, 1, 3), sl_min             # [2, L, O, 8]


def _prep_weights(weight):
    """-> xb_dev [2, 128, 5T, 128] fp8, wred [128, T, 16, 128] bf16, T."""
    cells, sl_min = _slice_cells(weight)
    n_act = SLICE_NUM - sl_min
    T = n_act                                   # col tiles of 128
    act = cells[:, :, :, sl_min:]               # [2, L, O, n_act]
    cols = act.reshape(2, L, O * n_act)         # col index c = o*n_act + k
    xb = cols.reshape(2, XR, XBAR, T, 128).astype(np.float32)

    xb_dev = np.zeros((2, 128, 5 * T, 128), np.float32)
    for i, r in enumerate(A_RS):
        for t in range(T):
            xb_dev[:, 0:64, i * T + t, :] = xb[:, r, :, t, :]
    for i, r in enumerate(B_RS):
        for t in range(T):
            xb_dev[:, 64:128, i * T + t, :] = xb[:, r, :, t, :]

    wred = np.zeros((128, T, STREAM_NUM, 128), np.float32)
    for t in range(T):
        for kk in range(128):
            c = 128 * t + kk
            o, k = divmod(c, n_act)
            sl = sl_min + k
            base = 2.0 ** (2 * (SLICE_NUM - 1 - sl))
            for s in range(STREAM_NUM):
                sw = 2.0 ** s * (-1.0 if s == STREAM_NUM - 1 else 1.0)
                wred[kk, t, s, o] = base * sw
    return (np.ascontiguousarray(xb_dev.astype(ml_dtypes.float8_e4m3)),
            np.ascontiguousarray(wred.astype(ml_dtypes.bfloat16)), T)


def _prep_bits(x):
    """-> [128, 5, 16, 2048] fp8: partition k<64 holds r=A_RS[j], k>=64 holds
    r=B_RS[j] (j the middle index; j=4 upper half unused)."""
    xp = np.pad(x, ((0, 0), (0, 0), (PAD, PAD), (PAD, PAD)))
    patches = np.stack([xp[:, :, di:di + OH, dj:dj + OW]
                        for di in range(KH) for dj in range(KW)], axis=2)
    feat = patches.reshape(B, L, OH * OW).transpose(0, 2, 1).reshape(P_TOTAL, L)
    x_int = np.clip(np.round(feat * 2.0**I_FRAC), -2**15, 2**15 - 1).astype(np.int32)
    x_u = np.where(x_int < 0, x_int + 2**16, x_int)
    shifts = np.arange(STREAM_NUM, dtype=np.int32)[:, None, None]
    bits = ((x_u[None] >> shifts) & 1).astype(np.float32)     # [16, 2048, 576]
    bits = bits.reshape(STREAM_NUM, P_TOTAL, XR, XBAR)        # [s, p, r, k]
    dev = np.zeros((128, 5, STREAM_NUM, P_TOTAL), np.float32)
    for j, r in enumerate(A_RS):
        dev[0:64, j] = bits[:, :, r, :].transpose(2, 0, 1)
    for j, r in enumerate(B_RS):
        dev[64:128, j] = bits[:, :, r, :].transpose(2, 0, 1)
    return np.ascontiguousarray(dev.astype(ml_dtypes.float8_e4m3))


# ------------------------------ bass program ------------------------------

def _build_nc(T, act_num=20, act_den=20, lag=3):
    """One SPMD program for all 8 cores.

    Units: 4T pair-units (A=(r=j,t) rows 0-63 + B=(r=4+j,t) rows 64-127,
    16 streams each) then T single-units (r=8).  Per unit-and-stream:
    pass1 matmul(s), one quantize op1 (ACT or DVE), one unbias op2 (DVE),
    then -- lagged by `lag` steps to keep the PE FIFO unblocked -- the
    pass2 accumulation matmuls.
    """
    import concourse.bass as bass
    import concourse.mybir as mybir
    import concourse.tile as tile

    f8 = mybir.dt.float8e4
    f16 = mybir.dt.float16
    bf16 = mybir.dt.bfloat16
    f32 = mybir.dt.float32

    SCALE = float(np.float32(85.0 / 64.0))
    BIAS = 1024.0

    nc = bass.Bass()
    bits_d = nc.dram_tensor("bits", [128, 5, STREAM_NUM, P_CORE], f8,
                            kind="ExternalInput")
    xb_d = nc.dram_tensor("xbars", [128, 5 * T, 128], f8, kind="ExternalInput")
    wred_d = nc.dram_tensor("wred", [128, T, STREAM_NUM, 128], bf16,
                            kind="ExternalInput")
    out_d = nc.dram_tensor("acc_out", [128, P_CORE], f32, kind="ExternalOutput")

    with ExitStack() as ctx:
        tc = ctx.enter_context(tile.TileContext(nc))
        singles = ctx.enter_context(tc.tile_pool(name="singles", bufs=1))
        # bufs >= lag+2 so buffer-recycle waits are implied by the PE FIFO
        # (see _strip_implied_waits) and can be dropped from ACT/DVE instrs.
        ypool_a = ctx.enter_context(tc.tile_pool(name="ya", bufs=lag + 2))
        ypool_b = ctx.enter_context(tc.tile_pool(name="yb", bufs=lag + 2))
        qpool = ctx.enter_context(tc.tile_pool(name="qp", bufs=lag + 3))
        opool = ctx.enter_context(tc.tile_pool(name="osb", bufs=1))
        psq_pool_a = ctx.enter_context(tc.tile_pool(name="psqa", bufs=2,
                                                    space="PSUM"))
        psq_pool_b = ctx.enter_context(tc.tile_pool(name="psqb", bufs=1,
                                                    space="PSUM"))
        psq_pool_s = ctx.enter_context(tc.tile_pool(name="psqs", bufs=1,
                                                    space="PSUM"))
        pacc_pool = ctx.enter_context(tc.tile_pool(name="pacc", bufs=1,
                                                   space="PSUM"))

        xb_sb = singles.tile([128, 5 * T, 128], f8)
        nc.default_dma_engine.dma_start(out=xb_sb[:], in_=xb_d[:, :, :])
        bits_sb = singles.tile([128, 5, STREAM_NUM, P_CORE], f8)
        nc.default_dma_engine.dma_start(out=bits_sb[:, 0, :, :],
                                        in_=bits_d[:, 0, :, :])
        wred_sb = singles.tile([128, T, STREAM_NUM, 128], bf16)
        nc.default_dma_engine.dma_start(out=wred_sb[:], in_=wred_d[:, :, :, :])
        for j in range(1, 5):
            nc.default_dma_engine.dma_start(out=bits_sb[:, j, :, :],
                                            in_=bits_d[:, j, :, :])

        acc = pacc_pool.tile([128, P_CORE], f32)

        # PE-side DMA fences: a standalone LDWEIGHTS reading each DMA'd
        # tensor carries that DMA's single wait; later PE instructions are
        # FIFO-ordered behind it, so the real matmuls never need to combine
        # a DMA wait with a buffer-recycle wait (walrus MM struct has only
        # one wait slot).
        def fence(ap):
            nc.tensor.ldweights(ap)

        fence(xb_sb[0:64, 0, :])
        fence(wred_sb[:, 0, 0, :])

        n_pass2 = 2 * 4 * T * STREAM_NUM + T * STREAM_NUM
        state = {"first": True, "done": 0}
        pending = []

        def emit_pass2(item):
            qp_t, t, s, paired = item
            halves = ((0, P_CORE), (P_CORE, 2 * P_CORE)) if paired \
                else ((0, P_CORE),)
            for lo, hi in halves:
                start = state["first"]
                state["first"] = False
                state["done"] += 1
                nc.tensor.matmul(acc[:, :], wred_sb[:, t, s, :],
                                 qp_t[:, lo:hi], start=start,
                                 stop=(state["done"] == n_pass2))

        uidx = 0
        for j in range(4):                       # paired units
            fence(bits_sb[0:64, j, 0, 0:128])
            for t in range(T):
                for s in range(STREAM_NUM):
                    use_act = (uidx % act_den) < act_num
                    pool = psq_pool_a if use_act else psq_pool_b
                    psq = pool.tile([128, 2 * P_CORE], f32,
                                    tag="pa" if use_act else "pb")
                    nc.tensor.matmul(psq[:, 0:P_CORE],
                                     xb_sb[0:64, j * T + t, :],
                                     bits_sb[0:64, j, s, :],
                                     start=True, stop=True,
                                     tile_position=(0, 0))
                    nc.tensor.matmul(psq[:, P_CORE:2 * P_CORE],
                                     xb_sb[64:128, j * T + t, :],
                                     bits_sb[64:128, j, s, :],
                                     start=True, stop=True,
                                     tile_position=(64, 0))
                    if use_act:
                        y = ypool_a.tile([128, 2 * P_CORE], f16, tag="ya")
                        nc.scalar.activation(
                            y[:, :], psq[:, :],
                            mybir.ActivationFunctionType.Copy,
                            bias=BIAS, scale=SCALE)
                    else:
                        y = ypool_b.tile([128, 2 * P_CORE], f16, tag="yb")
                        nc.vector.tensor_scalar(
                            y[:, :], psq[:, :], SCALE, BIAS,
                            mybir.AluOpType.mult, mybir.AluOpType.add)
                    qp = qpool.tile([128, 2 * P_CORE], bf16, tag="qp")
                    nc.vector.tensor_scalar(
                        qp[:, :], y[:, :], BIAS, None, mybir.AluOpType.subtract)
                    pending.append((qp, t, s, True))
                    if len(pending) > lag:
                        emit_pass2(pending.pop(0))
                    uidx += 1
        fence(bits_sb[0:64, 4, 0, 0:128])
        for t in range(T):                       # single units (r=8)
            for s in range(STREAM_NUM):
                psq = psq_pool_s.tile([128, P_CORE], f32, tag="ps")
                nc.tensor.matmul(psq[:, :], xb_sb[0:64, 4 * T + t, :],
                                 bits_sb[0:64, 4, s, :],
                                 start=True, stop=True, tile_position=(0, 0))
                y = ypool_a.tile([128, P_CORE], f16, tag="ys")
                nc.scalar.activation(
                    y[:, :], psq[:, :],
                    mybir.ActivationFunctionType.Copy, bias=BIAS, scale=SCALE)
                qp = qpool.tile([128, P_CORE], bf16, tag="qs")
                nc.vector.tensor_scalar(
                    qp[:, :], y[:, :], BIAS, None, mybir.AluOpType.subtract)
                pending.append((qp, t, s, False))
                if len(pending) > lag:
                    emit_pass2(pending.pop(0))
        while pending:
            emit_pass2(pending.pop(0))

        out_sb = opool.tile([128, P_CORE], f32)
        nc.vector.tensor_copy(out_sb[:, :], acc[:, :])
        nc.default_dma_engine.dma_start(out=out_d[:, :], in_=out_sb[:, :])

    _strip_own_engine_waits(nc, mybir)
    _strip_implied_waits(nc, mybir)
    return nc


def _strip_implied_waits(nc, mybir):
    """Walrus's ACT (and DVE) sync structs hold only one wait slot.  After
    own-engine stripping, the remaining 2-wait cases are data-ready plus
    buffer-recycle.  The recycle waits are implied transitively: every ACT
    op1 waits on its PE pass1 matmul, which sits in the PE FIFO after the
    pass2 matmul of `lag` units earlier, which waited on the DVE op2 of that
    unit -- so with pool depth >= lag+2 the recycled buffer's consumer is
    already done.  Rules:
      - InstActivation: keep only PE_* waits.
      - DVE InstTensorScalarPtr with an Activation_* wait: drop PE_* waits
        (the qp recycle; implied the same way through the ACT op1's PE wait).
    """
    for f in nc.m.functions:
        for b in f.blocks:
            for inst in b.instructions:
                si = getattr(inst, "sync_info", None)
                if si is None or not si.on_wait or len(si.on_wait) < 2:
                    continue
                ty = type(inst).__name__
                names = [str(w.ant_name) for w in si.on_wait]
                if ty == "InstActivation":
                    kept = [w for w in si.on_wait
                            if str(w.ant_name).startswith("PE_")]
                elif (ty == "InstTensorScalarPtr"
                      and any(n.startswith("Activation_") for n in names)):
                    kept = [w for w in si.on_wait
                            if not str(w.ant_name).startswith("PE_")]
                else:
                    continue
                if kept and len(kept) < len(si.on_wait):
                    inst.sync_info = mybir.SyncInfo(
                        on_wait=kept, on_update=list(si.on_update or []))


def _strip_own_engine_waits(nc, mybir):
    """Drop redundant same-engine semaphore waits (compute engines execute
    their queue serially, so ordering vs. their own past instructions is
    implicit).  Walrus's per-instruction sync structs have very few wait
    slots and reject Tile's extra own-engine waits."""
    eng_prefix = {
        "EngineType.PE": "PE",
        "EngineType.Activation": "Activation",
        "EngineType.DVE": "DVE",
        "EngineType.Pool": "Pool",
    }
    # The tail drain waits on every engine + every DMA queue, exceeding the
    # CTRL struct's wait slots.  All but the final DVE->DRAM chain are implied
    # transitively, so keep only the output DMA queue's semaphore.
    last_dma_sems = set()
    for f in nc.m.functions:
        for b in f.blocks:
            for inst in b.instructions:
                if type(inst).__name__ == "InstDMACopy" and inst.sync_info:
                    last_dma_sems = {str(w.ant_name)
                                     for w in (inst.sync_info.on_update or [])}
    for f in nc.m.functions:
        for b in f.blocks:
            for inst in b.instructions:
                si = getattr(inst, "sync_info", None)
                if (type(inst).__name__ == "InstDrain" and si and si.on_wait
                        and len(si.on_wait) > 2):
                    kept = [w for w in si.on_wait
                            if str(w.ant_name) in last_dma_sems]
                    inst.sync_info = mybir.SyncInfo(
                        on_wait=kept, on_update=list(si.on_update or []))
    for f in nc.m.functions:
        for b in f.blocks:
            for inst in b.instructions:
                si = getattr(inst, "sync_info", None)
                if si is None or not si.on_wait:
                    continue
                pfx = eng_prefix.get(str(getattr(inst, "engine", None)))
                if pfx is None:
                    continue
                kept = [w for w in si.on_wait
                        if not str(w.ant_name).startswith(pfx + "_")]
                if len(kept) != len(si.on_wait):
                    inst.sync_info = mybir.SyncInfo(
                        on_wait=kept, on_update=list(si.on_update or []))


def _get_nc(T):
    key = ("nc", T)
    if key not in _COMPILED:
        _COMPILED[key] = _build_nc(T)
    return _COMPILED[key]


# ------------------------------- entry point -------------------------------

def _make_in_maps(x, weight):
    xb_dev, wred, T = _prep_weights(weight)   # [2,128,5T,128], [128,T,16,128]
    bits = _prep_bits(x)                      # [128, 5, 16, 2048]
    in_maps = []
    for core in range(N_CORES):
        n, q = core // 4, core % 4
        in_maps.append({
            "bits": np.ascontiguousarray(
                bits[:, :, :, q * P_CORE:(q + 1) * P_CORE]),
            "xbars": np.ascontiguousarray(xb_dev[n]),
            "wred": wred,
        })
    return in_maps, T


def _postprocess(accs):
    """accs: list of 8 [128, 512] f32 arrays (core order) -> [8,128,16,16]."""
    acc_pos = np.concatenate([accs[q] for q in range(4)], axis=1)
    acc_neg = np.concatenate([accs[4 + q] for q in range(4)], axis=1)
    d32 = np.float32(192.0 / 255.0)
    out = ((acc_pos - acc_neg).astype(np.float32) * d32).astype(np.float32)
    out = out * np.float32(2.0**-24)
    amax = np.float32((2**15 - 1) / 2.0**12)
    out = np.clip(np.round(out * np.float32(4096.0)) / np.float32(4096.0),
                  -amax, amax).astype(np.float32)
    return np.ascontiguousarray(
        out.reshape(O, B, OH, OW).transpose(1, 0, 2, 3))


def run_on_hw(x, weight, trace=False):
    from concourse.bass_utils import run_bass_kernel_spmd
    in_maps, T = _make_in_maps(np.asarray(x, np.float32),
                               np.asarray(weight, np.float32))
    nc = _get_nc(T)
    res = run_bass_kernel_spmd(nc, in_maps, list(range(N_CORES)), trace=trace)
    accs = [np.asarray(res.results[i]["acc_out"], np.float32)
            for i in range(N_CORES)]
    return _postprocess(accs), res


def kernel(x, weight):
    out, _ = run_on_hw(x, weight, trace=False)
    return out
